# revision 2
# baseline (speedup 1.0000x reference)
"""Trainium2 Bass kernel for nn_CGLayer — v2: m-split + AllGather, product
message-passing on TensorE, bf16 end-to-end.

Device math per core (b, h):
  sx[m,g]   = sum_j s[m,g,j]                 fold-tree, own m-half, bf16
  (2-rank AllGather x2 exchanges sx halves; output is global-tile-ordered,
   so the program is identical on every core)
  q[m,p,c]  = sx[m,g(p)] * v9[m,v(p),c]      71 used products, bf16
  h[i,p,c]  = sum_m adjT[m,i] * q[m,p,c]     i in own half; two phases over
                                             m-tiles {0,1,4,5} / {2,3,6,7}
                                             bridged by identity matmuls
Host: mp[r] = sum_p CG[r,p] h[p] (fixed 51x71 mix), then the per-degree
Frobenius normalization — both tiny epilogues on the gathered output.
"""

import numpy as np
from math import factorial

import ml_dtypes

from concourse import bacc, tile, mybir
from concourse.bass_utils import run_bass_kernel_spmd

B, N, C = 4, 1024, 64
HALF = N // 2
NCORES = 8
LOFF = [0, 1, 4]

AluOp = mybir.AluOpType
dt = mybir.dt
ACT = mybir.ActivationFunctionType
BF16 = ml_dtypes.bfloat16


# ---------------------------------------------------------------- CG tables
def _cg_coeff(l1, m1, l2, m2, L, M):
    if m1 + m2 != M or not (abs(l1 - l2) <= L <= l1 + l2):
        return 0.0
    f = factorial
    pre = ((2 * L + 1) * f(L + l1 - l2) * f(L - l1 + l2) * f(l1 + l2 - L)
           / f(l1 + l2 + L + 1)) ** 0.5
    pre *= (f(L + M) * f(L - M) * f(l1 - m1) * f(l1 + m1) * f(l2 - m2)
            * f(l2 + m2)) ** 0.5
    s = 0.0
    for k in range(0, l1 + l2 - L + 1):
        dens = [k, l1 + l2 - L - k, l1 - m1 - k, l2 + m2 - k,
                L - l2 + m1 + k, L - l1 - m2 + k]
        if any(d < 0 for d in dens):
            continue
        term = (-1.0) ** k
        for d in dens:
            term /= f(d)
        s += term
    return pre * s


def _build_tables():
    rows = []
    for L in range(3):
        frags = [(l1, l2) for l1 in range(3) for l2 in range(3)
                 if abs(l1 - l2) <= L <= l1 + l2]
        for k in range(2 * L + 1):
            for (l1, l2) in frags:
                rows.append((L, k, l1, l2))
    entries = []
    for (L, k, l1, l2) in rows:
        M = k - L
        es = []
        for i in range(2 * l1 + 1):
            m1 = i - l1
            m2 = M - m1
            if abs(m2) <= l2:
                c = _cg_coeff(l1, m1, l2, m2, L, M)
                if c != 0.0:
                    es.append((LOFF[l1] + i, LOFF[l2] + l2 + m2, c))
        entries.append(es)
    return rows, entries


ROWS, ENTRIES = _build_tables()
NROWS = len(ROWS)                                   # 51
NCH = NROWS * C
_L_NROWS = [sum(1 for r in ROWS if r[0] == L) for L in range(3)]
L_RANGES = []
_c0 = 0
for L in range(3):
    L_RANGES.append((_c0, _c0 + _L_NROWS[L] * C))
    _c0 += _L_NROWS[L] * C

PRODS = sorted({(g, v) for es in ENTRIES for (v, g, cf) in es})
PIDX = {gv: i for i, gv in enumerate(PRODS)}
NPROD = len(PRODS)                                  # 71
W = NPROD * C                                       # 4544
CGM = np.zeros((NROWS, NPROD), np.float32)          # host combine matrix
for r, es in enumerate(ENTRIES):
    for (v, g, cf) in es:
        CGM[r, PIDX[(g, v)]] += cf

SEGS = []                       # (g, va, vb, p0): products p0.. = (g, va..vb)
for g in range(9):
    vs = sorted(v for (gg, v) in PRODS if gg == g)
    run = [vs[0]]
    for v in vs[1:]:
        if v == run[-1] + 1:
            run.append(v)
        else:
            SEGS.append((g, run[0], run[-1] + 1, PIDX[(g, run[0])]))
            run = [v]
    SEGS.append((g, run[0], run[-1] + 1, PIDX[(g, run[0])]))

NIC = HALF // 128                                   # 4 receiver tiles
PH1_T = [0, 4, 1, 5]            # global m-tiles available after AG1,
PH2_T = [2, 3, 6, 7]            # ordered by stage-1 readiness
CHUNK_USES = [(0, 2048), (2048, 2048), (4096, W - 4096)]   # psum-tile uses
SCAL_SEGS = {1, 2, 5, 6, 7, 8, 12}      # stage-1 segs on ScalarE (rest DVE)


# ---------------------------------------------------------------- program
def build():
    nc = bacc.Bacc("TRN2", target_bir_lowering=False, debug=False,
                   num_devices=NCORES)
    s_in = nc.dram_tensor("sh", [HALF, N, 9], dt.bfloat16, kind="ExternalInput")
    v9_in = nc.dram_tensor("v9", [N, 9, C], dt.bfloat16, kind="ExternalInput")
    adjT_in = nc.dram_tensor("adjT", [N, HALF], dt.bfloat16, kind="ExternalInput")
    id_in = nc.dram_tensor("ident", [128, 128], dt.bfloat16, kind="ExternalInput")
    h_out = nc.dram_tensor("h", [HALF, W], dt.bfloat16, kind="ExternalOutput")
    agx_in = nc.dram_tensor("agx_in", [4 * 128, 9], dt.float32)
    agx_out = nc.dram_tensor("agx_out", [8 * 128, 9], dt.float32)
    groups = [[2 * b, 2 * b + 1] for b in range(B)]

    with tile.TileContext(nc) as tc:
        with (tc.tile_pool(name="const", bufs=1) as cpool,
              tc.tile_pool(name="stream", bufs=2) as spool,
              tc.tile_pool(name="psum", bufs=1, space="PSUM") as pspool):
            adjT_sb = cpool.tile([128, 8, HALF], dt.bfloat16)
            v9_sb = cpool.tile([128, 8, 9, C], dt.bfloat16)
            id_sb = cpool.tile([128, 128], dt.bfloat16)
            sxp = cpool.tile([128, 4, 9], dt.float32)
            sx8 = cpool.tile([128, 8, 9], dt.float32)
            q = cpool.tile([128, 8, NPROD, C], dt.bfloat16)
            acc = cpool.tile([128, NIC, W], dt.bfloat16)

            qf = q.rearrange("p t a b -> p t (a b)")
            v9f = v9_sb.rearrange("p t a b -> p t (a b)")
            HN = (N // 2) * 9                      # flat fold-half size



            # ---- DMA: s tile in two j-halves (lower fold latency)
            def dma_s(t):
                st = spool.tile([128, N, 9], dt.bfloat16, tag="s", name=f"s{t}")
                stf = st.rearrange("p j g -> p (j g)")
                rows = slice(t * 128, (t + 1) * 128)
                nc.sync.dma_start(st[:, 0:N // 2, :], s_in[rows, 0:N // 2, :])
                nc.sync.dma_start(st[:, N // 2:N, :], s_in[rows, N // 2:N, :])
                return stf

            def fold(stf, t):
                # j-major layout: halving the flat array sums j-pairs per g
                for base in (0, HN):
                    half = HN // 2
                    while half >= 9:
                        nc.vector.tensor_tensor(
                            stf[:, base:base + half],
                            stf[:, base:base + half],
                            stf[:, base + half:base + 2 * half], op=AluOp.add)
                        half //= 2
                nc.vector.tensor_tensor(sxp[:, t, :], stf[:, 0:9],
                                        stf[:, HN:HN + 9], op=AluOp.add)

            sts = {}
            for t in (0, 1):
                sts[t] = dma_s(t)
            nc.sync.dma_start(id_sb[:, :], id_in[:, :])
            for t in range(8):
                nc.sync.dma_start(v9_sb[:, t, :, :],
                                  v9_in[t * 128:(t + 1) * 128, :, :])
            for t in range(8):
                nc.sync.dma_start(adjT_sb[:, t, :],
                                  adjT_in[t * 128:(t + 1) * 128, :])
            for t in (2, 3):
                sts[t] = dma_s(t)

            for t in (0, 1):
                fold(sts[t], t)
            for t in (2, 3):
                fold(sts[t], t)
            # single AllGather: own sx tiles -> global order [h0 4t | h1 4t]
            nc.gpsimd.dma_start(
                agx_in.rearrange("(t p) c -> p t c", p=128), sxp[:, 0:4, :])
            nc.gpsimd.collective_compute(
                "AllGather", AluOp.bypass, replica_groups=groups,
                ins=[agx_in[:]], outs=[agx_out[:]])
            vx = agx_out.rearrange("(t p) c -> p t c", p=128)
            nc.scalar.dma_start(sx8[:, 0:8, :], vx[:, 0:8, :])

            # ---- stage 1: q[t] = sx8[g] * v9[t][v], one engine per tile
            def stage1(t, eng):
                for (g, va, vb, p0) in SEGS:
                    w = (vb - va) * C
                    if eng == "scalar":
                        nc.scalar.activation(
                            qf[:, t, p0 * C:p0 * C + w],
                            v9f[:, t, va * C:vb * C], ACT.Copy,
                            scale=sx8[:, t, g:g + 1])
                    else:
                        nc.vector.tensor_scalar_mul(
                            qf[:, t, p0 * C:p0 * C + w],
                            v9f[:, t, va * C:vb * C], sx8[:, t, g:g + 1])

            # tiles split across engines so q streams out in t order
            for t in range(8):
                stage1(t, "vector" if t % 2 == 0 else "scalar")

            # ---- single-pass matmuls: all 8 m-tiles per psum lifetime
            for ic in range(NIC):
                for (c0, cw) in CHUNK_USES:
                    ps = pspool.tile([128, 2048], dt.float32,
                                     name=f"ps_{ic}_{c0}",
                                     tag=f"bk{(ic * 3 + c0 // 2048) % 2}")
                    nsl = (cw + 511) // 512
                    for t in range(8):
                        for k in range(nsl):
                            f0 = k * 512
                            fw = min(512, cw - f0)
                            nc.tensor.matmul(
                                ps[:, f0:f0 + fw],
                                adjT_sb[:, t, ic * 128:(ic + 1) * 128],
                                qf[:, t, c0 + f0:c0 + f0 + fw],
                                start=(t == 0), stop=(t == 7))
                    nc.scalar.activation(acc[:, ic, c0:c0 + cw], ps[:, 0:cw],
                                         ACT.Copy)
                nc.sync.dma_start(h_out[ic * 128:(ic + 1) * 128, :],
                                  acc[:, ic, :])
    nc.compile()
    return nc


_programs = {}


def _get_program():
    if "v2" not in _programs:
        _programs["v2"] = build()
    return _programs["v2"]


# ---------------------------------------------------------------- host driver
def kernel(v0, v1, v2, s0, s1, s2, conn, _trace=False, _results=None):
    v9 = np.concatenate([np.asarray(v0, np.float32),
                         np.asarray(v1, np.float32),
                         np.asarray(v2, np.float32)], axis=2).astype(BF16)
    s9 = np.concatenate(
        [np.asarray(s, np.float32)[..., 0] for s in (s0, s1, s2)],
        axis=3).astype(BF16)                                 # [B, m, j, 9]
    adjT = np.asarray(conn).transpose(0, 2, 1).astype(BF16)  # [B, m, i]
    ident = np.eye(128, dtype=np.float32).astype(BF16)

    core_ids = list(range(NCORES))
    in_maps = []
    for k in core_ids:
        b, h = divmod(k, 2)
        isl = slice(h * HALF, (h + 1) * HALF)
        in_maps.append({
            "sh": np.ascontiguousarray(s9[b, isl]),
            "v9": v9[b],
            "adjT": np.ascontiguousarray(adjT[b, :, isl]),
            "ident": ident,
        })

    r = run_bass_kernel_spmd(_get_program(), in_maps, core_ids, trace=_trace)
    h_all = np.empty((B, N, NPROD, C), np.float32)
    for k in core_ids:
        b, hh = divmod(k, 2)
        h_all[b, hh * HALF:(hh + 1) * HALF] = (
            r.results[k]["h"].astype(np.float32).reshape(HALF, NPROD, C))

    if _results is not None:
        _results.append(r)

    # host epilogue: fixed CG mix + per-degree normalization
    mp = np.einsum("rp,bipc->birc", CGM, h_all).reshape(B, N, NCH)
    out = np.empty_like(mp)
    for L, (c0, c1) in enumerate(L_RANGES):
        seg = mp[:, :, c0:c1]
        nf = (2 * L + 1) * np.linalg.norm(seg.astype(np.float64))
        out[:, :, c0:c1] = (seg.astype(np.float64) / (nf / C)).astype(np.float32)
    return out


# revision 3
# speedup vs baseline: 1.1170x; 1.1170x over previous
"""Trainium2 Bass kernel for nn_CGLayer — v2: m-split + AllGather, product
message-passing on TensorE, bf16 end-to-end.

Device math per core (b, h):
  sx[m,g]   = sum_j s[m,g,j]                 fold-tree, own m-half, bf16
  (2-rank AllGather x2 exchanges sx halves; output is global-tile-ordered,
   so the program is identical on every core)
  q[m,p,c]  = sx[m,g(p)] * v9[m,v(p),c]      71 used products, bf16
  h[i,p,c]  = sum_m adjT[m,i] * q[m,p,c]     i in own half; two phases over
                                             m-tiles {0,1,4,5} / {2,3,6,7}
                                             bridged by identity matmuls
Host: mp[r] = sum_p CG[r,p] h[p] (fixed 51x71 mix), then the per-degree
Frobenius normalization — both tiny epilogues on the gathered output.
"""

import numpy as np
from math import factorial

import ml_dtypes

from concourse import bacc, tile, mybir
from concourse.bass_utils import run_bass_kernel_spmd

B, N, C = 4, 1024, 64
HALF = N // 2
NCORES = 8
LOFF = [0, 1, 4]

AluOp = mybir.AluOpType
dt = mybir.dt
ACT = mybir.ActivationFunctionType
BF16 = ml_dtypes.bfloat16


# ---------------------------------------------------------------- CG tables
def _cg_coeff(l1, m1, l2, m2, L, M):
    if m1 + m2 != M or not (abs(l1 - l2) <= L <= l1 + l2):
        return 0.0
    f = factorial
    pre = ((2 * L + 1) * f(L + l1 - l2) * f(L - l1 + l2) * f(l1 + l2 - L)
           / f(l1 + l2 + L + 1)) ** 0.5
    pre *= (f(L + M) * f(L - M) * f(l1 - m1) * f(l1 + m1) * f(l2 - m2)
            * f(l2 + m2)) ** 0.5
    s = 0.0
    for k in range(0, l1 + l2 - L + 1):
        dens = [k, l1 + l2 - L - k, l1 - m1 - k, l2 + m2 - k,
                L - l2 + m1 + k, L - l1 - m2 + k]
        if any(d < 0 for d in dens):
            continue
        term = (-1.0) ** k
        for d in dens:
            term /= f(d)
        s += term
    return pre * s


def _build_tables():
    rows = []
    for L in range(3):
        frags = [(l1, l2) for l1 in range(3) for l2 in range(3)
                 if abs(l1 - l2) <= L <= l1 + l2]
        for k in range(2 * L + 1):
            for (l1, l2) in frags:
                rows.append((L, k, l1, l2))
    entries = []
    for (L, k, l1, l2) in rows:
        M = k - L
        es = []
        for i in range(2 * l1 + 1):
            m1 = i - l1
            m2 = M - m1
            if abs(m2) <= l2:
                c = _cg_coeff(l1, m1, l2, m2, L, M)
                if c != 0.0:
                    es.append((LOFF[l1] + i, LOFF[l2] + l2 + m2, c))
        entries.append(es)
    return rows, entries


ROWS, ENTRIES = _build_tables()
NROWS = len(ROWS)                                   # 51
NCH = NROWS * C
_L_NROWS = [sum(1 for r in ROWS if r[0] == L) for L in range(3)]
L_RANGES = []
_c0 = 0
for L in range(3):
    L_RANGES.append((_c0, _c0 + _L_NROWS[L] * C))
    _c0 += _L_NROWS[L] * C

PRODS = sorted({(g, v) for es in ENTRIES for (v, g, cf) in es})
PIDX = {gv: i for i, gv in enumerate(PRODS)}
NPROD = len(PRODS)                                  # 71
W = NPROD * C                                       # 4544
CGM = np.zeros((NROWS, NPROD), np.float32)          # host combine matrix
for r, es in enumerate(ENTRIES):
    for (v, g, cf) in es:
        CGM[r, PIDX[(g, v)]] += cf

SEGS = []                       # (g, va, vb, p0): products p0.. = (g, va..vb)
for g in range(9):
    vs = sorted(v for (gg, v) in PRODS if gg == g)
    run = [vs[0]]
    for v in vs[1:]:
        if v == run[-1] + 1:
            run.append(v)
        else:
            SEGS.append((g, run[0], run[-1] + 1, PIDX[(g, run[0])]))
            run = [v]
    SEGS.append((g, run[0], run[-1] + 1, PIDX[(g, run[0])]))

NIC = HALF // 128                                   # 4 receiver tiles
PH1_T = [0, 4, 1, 5]            # global m-tiles available after AG1,
PH2_T = [2, 3, 6, 7]            # ordered by stage-1 readiness
CHUNK_USES = [(0, 2048), (2048, 2048), (4096, W - 4096)]   # psum-tile uses
SCAL_SEGS = {1, 2, 5, 6, 7, 8, 12}      # stage-1 segs on ScalarE (rest DVE)


# ---------------------------------------------------------------- program
def build():
    nc = bacc.Bacc("TRN2", target_bir_lowering=False, debug=False,
                   num_devices=NCORES)
    s_in = nc.dram_tensor("sh", [HALF, N, 9], dt.bfloat16, kind="ExternalInput")
    v9_in = nc.dram_tensor("v9", [N, 9, C], dt.bfloat16, kind="ExternalInput")
    adjT_in = nc.dram_tensor("adjT", [N, HALF], dt.bfloat16, kind="ExternalInput")
    id_in = nc.dram_tensor("ident", [128, 128], dt.bfloat16, kind="ExternalInput")
    h_out = nc.dram_tensor("h", [HALF, W], dt.bfloat16, kind="ExternalOutput")
    ag_in = [nc.dram_tensor(f"ag_in{i}", [2 * 128, 9], dt.float32)
             for i in range(2)]
    ag_out = [nc.dram_tensor(f"ag_out{i}", [4 * 128, 9], dt.float32)
              for i in range(2)]
    groups = [[2 * b, 2 * b + 1] for b in range(B)]

    with tile.TileContext(nc) as tc:
        with (tc.tile_pool(name="const", bufs=1) as cpool,
              tc.tile_pool(name="stream", bufs=2) as spool,
              tc.tile_pool(name="psum", bufs=1, space="PSUM") as pspool):
            adjT_sb = cpool.tile([128, 8, HALF], dt.bfloat16)
            v9_sb = cpool.tile([128, 8, 9, C], dt.bfloat16)
            id_sb = cpool.tile([128, 128], dt.bfloat16)
            sxp = cpool.tile([128, 4, 9], dt.float32)
            sx8 = cpool.tile([128, 8, 9], dt.float32)
            q = cpool.tile([128, 8, NPROD, C], dt.bfloat16)
            acc = cpool.tile([128, NIC, W], dt.bfloat16)

            qf = q.rearrange("p t a b -> p t (a b)")
            v9f = v9_sb.rearrange("p t a b -> p t (a b)")
            HN = (N // 2) * 9                      # flat fold-half size



            # ---- DMA: s tile in two j-halves (lower fold latency)
            def dma_s(t):
                st = spool.tile([128, N, 9], dt.bfloat16, tag="s", name=f"s{t}")
                stf = st.rearrange("p j g -> p (j g)")
                rows = slice(t * 128, (t + 1) * 128)
                nc.sync.dma_start(st[:, 0:N // 2, :], s_in[rows, 0:N // 2, :])
                nc.sync.dma_start(st[:, N // 2:N, :], s_in[rows, N // 2:N, :])
                return stf

            def fold(stf, t):
                # j-major layout: halving the flat array sums j-pairs per g
                for base in (0, HN):
                    half = HN // 2
                    while half >= 9:
                        nc.vector.tensor_tensor(
                            stf[:, base:base + half],
                            stf[:, base:base + half],
                            stf[:, base + half:base + 2 * half], op=AluOp.add)
                        half //= 2
                nc.vector.tensor_tensor(sxp[:, t, :], stf[:, 0:9],
                                        stf[:, HN:HN + 9], op=AluOp.add)

            sts = {}
            for t in (0, 1):
                sts[t] = dma_s(t)
            nc.sync.dma_start(id_sb[:, :], id_in[:, :])
            for t in range(8):
                nc.sync.dma_start(v9_sb[:, t, :, :],
                                  v9_in[t * 128:(t + 1) * 128, :, :])
            for t in range(8):
                nc.sync.dma_start(adjT_sb[:, t, :],
                                  adjT_in[t * 128:(t + 1) * 128, :])
            for t in (2, 3):
                sts[t] = dma_s(t)

            for t in (0, 1):
                fold(sts[t], t)
            # AG1: own tiles {0,1} -> global {0,1} (rank0) / {4,5} (rank1)
            nc.gpsimd.dma_start(
                ag_in[0].rearrange("(t p) c -> p t c", p=128), sxp[:, 0:2, :])
            nc.gpsimd.collective_compute(
                "AllGather", AluOp.bypass, replica_groups=groups,
                ins=[ag_in[0][:]], outs=[ag_out[0][:]])
            v0 = ag_out[0].rearrange("(t p) c -> p t c", p=128)
            nc.scalar.dma_start(sx8[:, 0:2, :], v0[:, 0:2, :])
            nc.scalar.dma_start(sx8[:, 4:6, :], v0[:, 2:4, :])

            # ---- stage 1: q[t] = sx8[g] * v9[t][v], one engine per tile
            def stage1(t, eng):
                for (g, va, vb, p0) in SEGS:
                    w = (vb - va) * C
                    if eng == "scalar":
                        nc.scalar.activation(
                            qf[:, t, p0 * C:p0 * C + w],
                            v9f[:, t, va * C:vb * C], ACT.Copy,
                            scale=sx8[:, t, g:g + 1])
                    else:
                        nc.vector.tensor_scalar_mul(
                            qf[:, t, p0 * C:p0 * C + w],
                            v9f[:, t, va * C:vb * C], sx8[:, t, g:g + 1])

            stage1(0, "vector")
            stage1(4, "scalar")
            stage1(1, "vector")
            stage1(5, "scalar")

            for t in (2, 3):
                fold(sts[t], t)
            nc.gpsimd.dma_start(
                ag_in[1].rearrange("(t p) c -> p t c", p=128), sxp[:, 2:4, :])
            nc.gpsimd.collective_compute(
                "AllGather", AluOp.bypass, replica_groups=groups,
                ins=[ag_in[1][:]], outs=[ag_out[1][:]])
            v1 = ag_out[1].rearrange("(t p) c -> p t c", p=128)
            nc.scalar.dma_start(sx8[:, 2:4, :], v1[:, 0:2, :])
            nc.scalar.dma_start(sx8[:, 6:8, :], v1[:, 2:4, :])

            # ---- phase 1 matmuls: m-tiles {0,4,1,5}
            def mm_phase(ic, c0, cw, ts, join):
                ps = pspool.tile([128, 2048], dt.float32,
                                 name=f"ps_{join}_{ic}_{c0}",
                                 tag=f"bk{(ic * 3 + c0 // 2048) % 2}")
                nsl = (cw + 511) // 512
                if join:
                    for k in range(nsl):
                        f0 = k * 512
                        fw = min(512, cw - f0)
                        nc.tensor.matmul(ps[:, f0:f0 + fw], id_sb[:, :],
                                         acc[:, ic, c0 + f0:c0 + f0 + fw],
                                         start=True, stop=False)
                for j, t in enumerate(ts):
                    last = j == len(ts) - 1
                    for k in range(nsl):
                        f0 = k * 512
                        fw = min(512, cw - f0)
                        nc.tensor.matmul(
                            ps[:, f0:f0 + fw],
                            adjT_sb[:, t, ic * 128:(ic + 1) * 128],
                            qf[:, t, c0 + f0:c0 + f0 + fw],
                            start=(j == 0 and not join),
                            stop=last)
                nc.scalar.activation(acc[:, ic, c0:c0 + cw], ps[:, 0:cw],
                                     ACT.Copy)

            for ic in range(NIC):
                for (c0, cw) in CHUNK_USES:
                    mm_phase(ic, c0, cw, PH1_T, join=False)

            stage1(2, "vector")
            stage1(3, "vector")
            stage1(6, "scalar")
            stage1(7, "vector")

            # ---- phase 2: identity-join + m-tiles {2,3,6,7}, then DMA out
            for ic in range(NIC):
                for (c0, cw) in CHUNK_USES:
                    mm_phase(ic, c0, cw, PH2_T, join=True)
                nc.sync.dma_start(h_out[ic * 128:(ic + 1) * 128, :],
                                  acc[:, ic, :])
    nc.compile()
    return nc


_programs = {}


def _get_program():
    if "v2" not in _programs:
        _programs["v2"] = build()
    return _programs["v2"]


# ---------------------------------------------------------------- host driver
def kernel(v0, v1, v2, s0, s1, s2, conn, _trace=False, _results=None):
    v9 = np.concatenate([np.asarray(v0, np.float32),
                         np.asarray(v1, np.float32),
                         np.asarray(v2, np.float32)], axis=2).astype(BF16)
    s9 = np.concatenate(
        [np.asarray(s, np.float32)[..., 0] for s in (s0, s1, s2)],
        axis=3).astype(BF16)                                 # [B, m, j, 9]
    adjT = np.asarray(conn).transpose(0, 2, 1).astype(BF16)  # [B, m, i]
    ident = np.eye(128, dtype=np.float32).astype(BF16)

    core_ids = list(range(NCORES))
    in_maps = []
    for k in core_ids:
        b, h = divmod(k, 2)
        isl = slice(h * HALF, (h + 1) * HALF)
        in_maps.append({
            "sh": np.ascontiguousarray(s9[b, isl]),
            "v9": v9[b],
            "adjT": np.ascontiguousarray(adjT[b, :, isl]),
            "ident": ident,
        })

    r = run_bass_kernel_spmd(_get_program(), in_maps, core_ids, trace=_trace)
    h_all = np.empty((B, N, NPROD, C), np.float32)
    for k in core_ids:
        b, hh = divmod(k, 2)
        h_all[b, hh * HALF:(hh + 1) * HALF] = (
            r.results[k]["h"].astype(np.float32).reshape(HALF, NPROD, C))

    if _results is not None:
        _results.append(r)

    # host epilogue: fixed CG mix + per-degree normalization
    mp = np.einsum("rp,bipc->birc", CGM, h_all).reshape(B, N, NCH)
    out = np.empty_like(mp)
    for L, (c0, c1) in enumerate(L_RANGES):
        seg = mp[:, :, c0:c1]
        nf = (2 * L + 1) * np.linalg.norm(seg.astype(np.float64))
        out[:, :, c0:c1] = (seg.astype(np.float64) / (nf / C)).astype(np.float32)
    return out


# revision 4
# speedup vs baseline: 1.1426x; 1.0230x over previous
"""Trainium2 Bass kernel for nn_CGLayer — v2: m-split + AllGather, product
message-passing on TensorE, bf16 end-to-end.

Device math per core (b, h):
  sx[m,g]   = sum_j s[m,g,j]                 fold-tree, own m-half, bf16
  (2-rank AllGather x2 exchanges sx halves; output is global-tile-ordered,
   so the program is identical on every core)
  q[m,p,c]  = sx[m,g(p)] * v9[m,v(p),c]      71 used products, bf16
  h[i,p,c]  = sum_m adjT[m,i] * q[m,p,c]     i in own half; two phases over
                                             m-tiles {0,1,4,5} / {2,3,6,7}
                                             bridged by identity matmuls
Host: mp[r] = sum_p CG[r,p] h[p] (fixed 51x71 mix), then the per-degree
Frobenius normalization — both tiny epilogues on the gathered output.
"""

import numpy as np
from math import factorial

import ml_dtypes

from concourse import bacc, tile, mybir
from concourse.bass_utils import run_bass_kernel_spmd

B, N, C = 4, 1024, 64
HALF = N // 2
NCORES = 8
LOFF = [0, 1, 4]

AluOp = mybir.AluOpType
dt = mybir.dt
ACT = mybir.ActivationFunctionType
BF16 = ml_dtypes.bfloat16


# ---------------------------------------------------------------- CG tables
def _cg_coeff(l1, m1, l2, m2, L, M):
    if m1 + m2 != M or not (abs(l1 - l2) <= L <= l1 + l2):
        return 0.0
    f = factorial
    pre = ((2 * L + 1) * f(L + l1 - l2) * f(L - l1 + l2) * f(l1 + l2 - L)
           / f(l1 + l2 + L + 1)) ** 0.5
    pre *= (f(L + M) * f(L - M) * f(l1 - m1) * f(l1 + m1) * f(l2 - m2)
            * f(l2 + m2)) ** 0.5
    s = 0.0
    for k in range(0, l1 + l2 - L + 1):
        dens = [k, l1 + l2 - L - k, l1 - m1 - k, l2 + m2 - k,
                L - l2 + m1 + k, L - l1 - m2 + k]
        if any(d < 0 for d in dens):
            continue
        term = (-1.0) ** k
        for d in dens:
            term /= f(d)
        s += term
    return pre * s


def _build_tables():
    rows = []
    for L in range(3):
        frags = [(l1, l2) for l1 in range(3) for l2 in range(3)
                 if abs(l1 - l2) <= L <= l1 + l2]
        for k in range(2 * L + 1):
            for (l1, l2) in frags:
                rows.append((L, k, l1, l2))
    entries = []
    for (L, k, l1, l2) in rows:
        M = k - L
        es = []
        for i in range(2 * l1 + 1):
            m1 = i - l1
            m2 = M - m1
            if abs(m2) <= l2:
                c = _cg_coeff(l1, m1, l2, m2, L, M)
                if c != 0.0:
                    es.append((LOFF[l1] + i, LOFF[l2] + l2 + m2, c))
        entries.append(es)
    return rows, entries


ROWS, ENTRIES = _build_tables()
NROWS = len(ROWS)                                   # 51
NCH = NROWS * C
_L_NROWS = [sum(1 for r in ROWS if r[0] == L) for L in range(3)]
L_RANGES = []
_c0 = 0
for L in range(3):
    L_RANGES.append((_c0, _c0 + _L_NROWS[L] * C))
    _c0 += _L_NROWS[L] * C

PRODS = sorted({(g, v) for es in ENTRIES for (v, g, cf) in es})
PIDX = {gv: i for i, gv in enumerate(PRODS)}
NPROD = len(PRODS)                                  # 71
W = NPROD * C                                       # 4544
CGM = np.zeros((NROWS, NPROD), np.float32)          # host combine matrix
for r, es in enumerate(ENTRIES):
    for (v, g, cf) in es:
        CGM[r, PIDX[(g, v)]] += cf

SEGS = []                       # (g, va, vb, p0): products p0.. = (g, va..vb)
for g in range(9):
    vs = sorted(v for (gg, v) in PRODS if gg == g)
    run = [vs[0]]
    for v in vs[1:]:
        if v == run[-1] + 1:
            run.append(v)
        else:
            SEGS.append((g, run[0], run[-1] + 1, PIDX[(g, run[0])]))
            run = [v]
    SEGS.append((g, run[0], run[-1] + 1, PIDX[(g, run[0])]))

NIC = HALF // 128                                   # 4 receiver tiles
PH1_T = [0, 1, 2, 3]            # local own m-tiles (no collective needed)
PH2_T = [4, 5, 6, 7]            # peer m-tiles via the single AllGather
CHUNK_USES = [(0, 2048), (2048, 2048), (4096, W - 4096)]   # psum-tile uses
SCAL_SEGS = {1, 2, 5, 6, 7, 8, 12}      # stage-1 segs on ScalarE (rest DVE)


# ---------------------------------------------------------------- program
def build():
    nc = bacc.Bacc("TRN2", target_bir_lowering=False, debug=False,
                   num_devices=NCORES)
    s_in = nc.dram_tensor("sh", [HALF, N, 9], dt.bfloat16, kind="ExternalInput")
    v9_in = nc.dram_tensor("v9", [N, 9, C], dt.bfloat16, kind="ExternalInput")
    adjT_in = nc.dram_tensor("adjT", [N, HALF], dt.bfloat16, kind="ExternalInput")
    id_in = nc.dram_tensor("ident", [128, 128], dt.bfloat16, kind="ExternalInput")
    h_out = nc.dram_tensor("h", [HALF, W], dt.bfloat16, kind="ExternalOutput")
    pm_in = nc.dram_tensor("pm", [128, 2], dt.float32, kind="ExternalInput")
    agx_in = nc.dram_tensor("agx_in", [4 * 128, 9], dt.float32)
    agx_out = nc.dram_tensor("agx_out", [8 * 128, 9], dt.float32)
    groups = [[2 * b, 2 * b + 1] for b in range(B)]

    with tile.TileContext(nc) as tc:
        with (tc.tile_pool(name="const", bufs=1) as cpool,
              tc.tile_pool(name="stream", bufs=2) as spool,
              tc.tile_pool(name="psum", bufs=1, space="PSUM") as pspool):
            adjT_sb = cpool.tile([128, 8, HALF], dt.bfloat16)
            v9_sb = cpool.tile([128, 8, 9, C], dt.bfloat16)
            id_sb = cpool.tile([128, 128], dt.bfloat16)
            sx8 = cpool.tile([128, 8, 9], dt.float32)
            sxg = cpool.tile([128, 2, 4, 9], dt.float32)
            pm_sb = cpool.tile([128, 2], dt.float32)
            q = cpool.tile([128, 8, NPROD, C], dt.bfloat16)
            acc = cpool.tile([128, NIC, W], dt.bfloat16)

            qf = q.rearrange("p t a b -> p t (a b)")
            v9f = v9_sb.rearrange("p t a b -> p t (a b)")
            HN = (N // 2) * 9                      # flat fold-half size



            # ---- DMA: s tile in two j-halves (lower fold latency)
            def dma_s(t):
                st = spool.tile([128, N, 9], dt.bfloat16, tag="s", name=f"s{t}")
                stf = st.rearrange("p j g -> p (j g)")
                rows = slice(t * 128, (t + 1) * 128)
                nc.sync.dma_start(st[:, 0:N // 2, :], s_in[rows, 0:N // 2, :])
                nc.sync.dma_start(st[:, N // 2:N, :], s_in[rows, N // 2:N, :])
                return stf

            def fold(stf, t):
                # j-major layout: halving the flat array sums j-pairs per g
                for base in (0, HN):
                    half = HN // 2
                    while half >= 9:
                        nc.vector.tensor_tensor(
                            stf[:, base:base + half],
                            stf[:, base:base + half],
                            stf[:, base + half:base + 2 * half], op=AluOp.add)
                        half //= 2
                nc.vector.tensor_tensor(sx8[:, t, :], stf[:, 0:9],
                                        stf[:, HN:HN + 9], op=AluOp.add)

            sts = {}
            for t in (0, 1):
                sts[t] = dma_s(t)
            nc.sync.dma_start(id_sb[:, :], id_in[:, :])
            nc.sync.dma_start(pm_sb[:, :], pm_in[:, :])
            for t in range(8):
                nc.sync.dma_start(v9_sb[:, t, :, :],
                                  v9_in[t * 128:(t + 1) * 128, :, :])
            for t in range(8):
                nc.sync.dma_start(adjT_sb[:, t, :],
                                  adjT_in[t * 128:(t + 1) * 128, :])
            for t in (2, 3):
                sts[t] = dma_s(t)

            for t in (0, 1, 2, 3):
                fold(sts[t], t)

            # ---- stage 1: q[t] = sx8[g] * v9[t][v], one engine per tile
            def stage1(t, eng):
                for (g, va, vb, p0) in SEGS:
                    w = (vb - va) * C
                    if eng == "scalar":
                        nc.scalar.activation(
                            qf[:, t, p0 * C:p0 * C + w],
                            v9f[:, t, va * C:vb * C], ACT.Copy,
                            scale=sx8[:, t, g:g + 1])
                    else:
                        nc.vector.tensor_scalar_mul(
                            qf[:, t, p0 * C:p0 * C + w],
                            v9f[:, t, va * C:vb * C], sx8[:, t, g:g + 1])

            # own tiles: sx is local (no collective) -> ScalarE builds q
            # while DVE folds the next tile
            for t in range(4):
                stage1(t, "scalar")

            # single AllGather: own sx -> [rank0 4 tiles | rank1 4 tiles]
            nc.gpsimd.dma_start(
                agx_in.rearrange("(t p) c -> p t c", p=128), sx8[:, 0:4, :])
            nc.gpsimd.collective_compute(
                "AllGather", AluOp.bypass, replica_groups=groups,
                ins=[agx_in[:]], outs=[agx_out[:]])
            # ---- message-passing matmuls
            use_ctr = [0]

            def mm_phase(ic, c0, cw, ts, join):
                ps = pspool.tile([128, 2048], dt.float32,
                                 name=f"ps_{join}_{ic}_{c0}",
                                 tag=f"bk{(ic * 3 + c0 // 2048) % 2}")
                nsl = (cw + 511) // 512
                if join:
                    for k in range(nsl):
                        f0 = k * 512
                        fw = min(512, cw - f0)
                        nc.tensor.matmul(ps[:, f0:f0 + fw], id_sb[:, :],
                                         acc[:, ic, c0 + f0:c0 + f0 + fw],
                                         start=True, stop=False)
                for j, t in enumerate(ts):
                    last = j == len(ts) - 1
                    for k in range(nsl):
                        f0 = k * 512
                        fw = min(512, cw - f0)
                        nc.tensor.matmul(
                            ps[:, f0:f0 + fw],
                            adjT_sb[:, t, ic * 128:(ic + 1) * 128],
                            qf[:, t, c0 + f0:c0 + f0 + fw],
                            start=(j == 0 and not join),
                            stop=last)
                if use_ctr[0] % 2 == 0:
                    nc.scalar.activation(acc[:, ic, c0:c0 + cw], ps[:, 0:cw],
                                         ACT.Copy)
                else:
                    nc.vector.tensor_copy(acc[:, ic, c0:c0 + cw], ps[:, 0:cw])
                use_ctr[0] += 1

            for ic in range(NIC):
                for (c0, cw) in CHUNK_USES:
                    mm_phase(ic, c0, cw, PH1_T, join=False)

            vx = agx_out.rearrange("(s t p) c -> p s t c", p=128, t=4)
            nc.scalar.dma_start(sxg[:, :, :, :], vx[:, :, :, :])
            # peer sx = shard0*h + shard1*(1-h)  (pm columns from host)
            nc.vector.tensor_scalar_mul(sx8[:, 4:8, :], sxg[:, 0, :, :],
                                        pm_sb[:, 0:1])
            nc.vector.scalar_tensor_tensor(sx8[:, 4:8, :], sxg[:, 1, :, :],
                                           pm_sb[:, 1:2], sx8[:, 4:8, :],
                                           op0=AluOp.mult, op1=AluOp.add)
            stage1(4, "vector")
            stage1(5, "scalar")
            stage1(6, "vector")
            stage1(7, "scalar")

            # ---- phase 2: identity-join + m-tiles {2,3,6,7}, then DMA out
            for ic in range(NIC):
                for (c0, cw) in CHUNK_USES:
                    mm_phase(ic, c0, cw, PH2_T, join=True)
                nc.sync.dma_start(h_out[ic * 128:(ic + 1) * 128, :],
                                  acc[:, ic, :])
    nc.compile()
    return nc


_programs = {}


def _get_program():
    if "v2" not in _programs:
        _programs["v2"] = build()
    return _programs["v2"]


# ---------------------------------------------------------------- host driver
def kernel(v0, v1, v2, s0, s1, s2, conn, _trace=False, _results=None):
    v9 = np.concatenate([np.asarray(v0, np.float32),
                         np.asarray(v1, np.float32),
                         np.asarray(v2, np.float32)], axis=2).astype(BF16)
    s9 = np.concatenate(
        [np.asarray(s, np.float32)[..., 0] for s in (s0, s1, s2)],
        axis=3).astype(BF16)                                 # [B, m, j, 9]
    adjT = np.asarray(conn).transpose(0, 2, 1).astype(BF16)  # [B, m, i]
    ident = np.eye(128, dtype=np.float32).astype(BF16)

    core_ids = list(range(NCORES))
    in_maps = []
    for k in core_ids:
        b, h = divmod(k, 2)
        isl = slice(h * HALF, (h + 1) * HALF)
        own = slice(h * HALF, (h + 1) * HALF)
        peer = slice((1 - h) * HALF, (2 - h) * HALF)
        pm = np.zeros((128, 2), np.float32)
        pm[:, 0] = h            # coefficient of rank-0 shard for the peer half
        pm[:, 1] = 1 - h        # coefficient of rank-1 shard
        in_maps.append({
            "sh": np.ascontiguousarray(s9[b, own]),
            "v9": np.ascontiguousarray(
                np.concatenate([v9[b, own], v9[b, peer]], axis=0)),
            "adjT": np.ascontiguousarray(
                np.concatenate([adjT[b, own, isl], adjT[b, peer, isl]],
                               axis=0)),
            "ident": ident,
            "pm": pm,
        })

    r = run_bass_kernel_spmd(_get_program(), in_maps, core_ids, trace=_trace)
    h_all = np.empty((B, N, NPROD, C), np.float32)
    for k in core_ids:
        b, hh = divmod(k, 2)
        h_all[b, hh * HALF:(hh + 1) * HALF] = (
            r.results[k]["h"].astype(np.float32).reshape(HALF, NPROD, C))

    if _results is not None:
        _results.append(r)

    # host epilogue: fixed CG mix + per-degree normalization
    mp = np.einsum("rp,bipc->birc", CGM, h_all).reshape(B, N, NCH)
    out = np.empty_like(mp)
    for L, (c0, c1) in enumerate(L_RANGES):
        seg = mp[:, :, c0:c1]
        nf = (2 * L + 1) * np.linalg.norm(seg.astype(np.float64))
        out[:, :, c0:c1] = (seg.astype(np.float64) / (nf / C)).astype(np.float32)
    return out


# revision 5
# speedup vs baseline: 1.1780x; 1.0310x over previous
"""Trainium2 Bass kernel for nn_CGLayer — v2: m-split + AllGather, product
message-passing on TensorE, bf16 end-to-end.

Device math per core (b, h):
  sx[m,g]   = sum_j s[m,g,j]                 fold-tree, own m-half, bf16
  (2-rank AllGather x2 exchanges sx halves; output is global-tile-ordered,
   so the program is identical on every core)
  q[m,p,c]  = sx[m,g(p)] * v9[m,v(p),c]      71 used products, bf16
  h[i,p,c]  = sum_m adjT[m,i] * q[m,p,c]     i in own half; two phases over
                                             m-tiles {0,1,4,5} / {2,3,6,7}
                                             bridged by identity matmuls
Host: mp[r] = sum_p CG[r,p] h[p] (fixed 51x71 mix), then the per-degree
Frobenius normalization — both tiny epilogues on the gathered output.
"""

import numpy as np
from math import factorial

import ml_dtypes

from concourse import bacc, tile, mybir
from concourse.bass_utils import run_bass_kernel_spmd

B, N, C = 4, 1024, 64
HALF = N // 2
NCORES = 8
LOFF = [0, 1, 4]

AluOp = mybir.AluOpType
dt = mybir.dt
ACT = mybir.ActivationFunctionType
BF16 = ml_dtypes.bfloat16


# ---------------------------------------------------------------- CG tables
def _cg_coeff(l1, m1, l2, m2, L, M):
    if m1 + m2 != M or not (abs(l1 - l2) <= L <= l1 + l2):
        return 0.0
    f = factorial
    pre = ((2 * L + 1) * f(L + l1 - l2) * f(L - l1 + l2) * f(l1 + l2 - L)
           / f(l1 + l2 + L + 1)) ** 0.5
    pre *= (f(L + M) * f(L - M) * f(l1 - m1) * f(l1 + m1) * f(l2 - m2)
            * f(l2 + m2)) ** 0.5
    s = 0.0
    for k in range(0, l1 + l2 - L + 1):
        dens = [k, l1 + l2 - L - k, l1 - m1 - k, l2 + m2 - k,
                L - l2 + m1 + k, L - l1 - m2 + k]
        if any(d < 0 for d in dens):
            continue
        term = (-1.0) ** k
        for d in dens:
            term /= f(d)
        s += term
    return pre * s


def _build_tables():
    rows = []
    for L in range(3):
        frags = [(l1, l2) for l1 in range(3) for l2 in range(3)
                 if abs(l1 - l2) <= L <= l1 + l2]
        for k in range(2 * L + 1):
            for (l1, l2) in frags:
                rows.append((L, k, l1, l2))
    entries = []
    for (L, k, l1, l2) in rows:
        M = k - L
        es = []
        for i in range(2 * l1 + 1):
            m1 = i - l1
            m2 = M - m1
            if abs(m2) <= l2:
                c = _cg_coeff(l1, m1, l2, m2, L, M)
                if c != 0.0:
                    es.append((LOFF[l1] + i, LOFF[l2] + l2 + m2, c))
        entries.append(es)
    return rows, entries


ROWS, ENTRIES = _build_tables()
NROWS = len(ROWS)                                   # 51
NCH = NROWS * C
_L_NROWS = [sum(1 for r in ROWS if r[0] == L) for L in range(3)]
L_RANGES = []
_c0 = 0
for L in range(3):
    L_RANGES.append((_c0, _c0 + _L_NROWS[L] * C))
    _c0 += _L_NROWS[L] * C

PRODS = sorted({(g, v) for es in ENTRIES for (v, g, cf) in es})
PIDX = {gv: i for i, gv in enumerate(PRODS)}
NPROD = len(PRODS)                                  # 71
W = NPROD * C                                       # 4544
CGM = np.zeros((NROWS, NPROD), np.float32)          # host combine matrix
for r, es in enumerate(ENTRIES):
    for (v, g, cf) in es:
        CGM[r, PIDX[(g, v)]] += cf

SEGS = []                       # (g, va, vb, p0): products p0.. = (g, va..vb)
for g in range(9):
    vs = sorted(v for (gg, v) in PRODS if gg == g)
    run = [vs[0]]
    for v in vs[1:]:
        if v == run[-1] + 1:
            run.append(v)
        else:
            SEGS.append((g, run[0], run[-1] + 1, PIDX[(g, run[0])]))
            run = [v]
    SEGS.append((g, run[0], run[-1] + 1, PIDX[(g, run[0])]))

NIC = HALF // 128                                   # 4 receiver tiles
PH1_T = [0, 1, 2, 3]            # local own m-tiles (no collective needed)
PH2_T = [4, 5, 6, 7]            # peer m-tiles via the single AllGather
CHUNK_USES = [(0, 2048), (2048, 2048), (4096, W - 4096)]   # psum-tile uses
SCAL_SEGS = {1, 2, 5, 6, 7, 8, 12}      # stage-1 segs on ScalarE (rest DVE)


# ---------------------------------------------------------------- program
def build():
    nc = bacc.Bacc("TRN2", target_bir_lowering=False, debug=False,
                   num_devices=NCORES)
    s_in = nc.dram_tensor("sh", [HALF, N, 9], dt.bfloat16, kind="ExternalInput")
    v9_in = nc.dram_tensor("v9", [N, 9, C], dt.bfloat16, kind="ExternalInput")
    adjT_in = nc.dram_tensor("adjT", [N, HALF], dt.bfloat16, kind="ExternalInput")
    id_in = nc.dram_tensor("ident", [128, 128], dt.bfloat16, kind="ExternalInput")
    h_out = nc.dram_tensor("h", [HALF, W], dt.bfloat16, kind="ExternalOutput")
    pm_in = nc.dram_tensor("pm", [128, 2], dt.float32, kind="ExternalInput")
    agx_in = nc.dram_tensor("agx_in", [4 * 128, 9], dt.float32)
    agx_out = nc.dram_tensor("agx_out", [8 * 128, 9], dt.float32)
    agd_in = nc.dram_tensor("agd_in", [2, 2], dt.float32)
    agd_out = nc.dram_tensor("agd_out", [4, 2], dt.float32)
    groups = [[2 * b, 2 * b + 1] for b in range(B)]

    with tile.TileContext(nc) as tc:
        with (tc.tile_pool(name="const", bufs=1) as cpool,
              tc.tile_pool(name="stream", bufs=2) as spool,
              tc.tile_pool(name="psum", bufs=1, space="PSUM") as pspool):
            adjT_sb = cpool.tile([128, 8, HALF], dt.bfloat16)
            v9_sb = cpool.tile([128, 8, 9, C], dt.bfloat16)
            id_sb = cpool.tile([128, 128], dt.bfloat16)
            sx8 = cpool.tile([128, 8, 9], dt.float32)
            sxg = cpool.tile([128, 2, 4, 9], dt.float32)
            pm_sb = cpool.tile([128, 2], dt.float32)
            q = cpool.tile([128, 8, NPROD, C], dt.bfloat16)
            acc = cpool.tile([128, NIC, W], dt.bfloat16)

            qf = q.rearrange("p t a b -> p t (a b)")
            v9f = v9_sb.rearrange("p t a b -> p t (a b)")
            HN = (N // 2) * 9                      # flat fold-half size

            # dummy collective at t=0: pays the ncfw first-op setup (~24us)
            # under the fold phase so the real AllGather runs promptly
            dum = cpool.tile([2, 2], dt.float32)
            nc.gpsimd.memset(dum[:, :], 0.0)
            nc.gpsimd.dma_start(agd_in[:, :], dum[:, :])
            nc.gpsimd.collective_compute(
                "AllGather", AluOp.bypass, replica_groups=groups,
                ins=[agd_in[:]], outs=[agd_out[:]])



            # ---- DMA: s tile in two j-halves (lower fold latency)
            def dma_s(t):
                st = spool.tile([128, N, 9], dt.bfloat16, tag="s", name=f"s{t}")
                stf = st.rearrange("p j g -> p (j g)")
                rows = slice(t * 128, (t + 1) * 128)
                nc.sync.dma_start(st[:, 0:N // 2, :], s_in[rows, 0:N // 2, :])
                nc.sync.dma_start(st[:, N // 2:N, :], s_in[rows, N // 2:N, :])
                return stf

            def fold(stf, t):
                # j-major layout: halving the flat array sums j-pairs per g
                for base in (0, HN):
                    half = HN // 2
                    while half >= 9:
                        nc.vector.tensor_tensor(
                            stf[:, base:base + half],
                            stf[:, base:base + half],
                            stf[:, base + half:base + 2 * half], op=AluOp.add)
                        half //= 2
                nc.vector.tensor_tensor(sx8[:, t, :], stf[:, 0:9],
                                        stf[:, HN:HN + 9], op=AluOp.add)

            sts = {}
            for t in (0, 1):
                sts[t] = dma_s(t)
            nc.sync.dma_start(id_sb[:, :], id_in[:, :])
            nc.sync.dma_start(pm_sb[:, :], pm_in[:, :])
            for t in range(8):
                nc.sync.dma_start(v9_sb[:, t, :, :],
                                  v9_in[t * 128:(t + 1) * 128, :, :])
            for t in range(8):
                nc.sync.dma_start(adjT_sb[:, t, :],
                                  adjT_in[t * 128:(t + 1) * 128, :])
            for t in (2, 3):
                sts[t] = dma_s(t)

            for t in (0, 1, 2, 3):
                fold(sts[t], t)

            # ---- stage 1: q[t] = sx8[g] * v9[t][v], one engine per tile
            def stage1(t, eng):
                for (g, va, vb, p0) in SEGS:
                    w = (vb - va) * C
                    if eng == "scalar":
                        nc.scalar.activation(
                            qf[:, t, p0 * C:p0 * C + w],
                            v9f[:, t, va * C:vb * C], ACT.Copy,
                            scale=sx8[:, t, g:g + 1])
                    else:
                        nc.vector.tensor_scalar_mul(
                            qf[:, t, p0 * C:p0 * C + w],
                            v9f[:, t, va * C:vb * C], sx8[:, t, g:g + 1])

            # own tiles: sx is local (no collective) -> ScalarE builds q
            # while DVE folds the next tile
            for t in range(4):
                stage1(t, "scalar")

            # single AllGather: own sx -> [rank0 4 tiles | rank1 4 tiles]
            nc.gpsimd.dma_start(
                agx_in.rearrange("(t p) c -> p t c", p=128), sx8[:, 0:4, :])
            nc.gpsimd.collective_compute(
                "AllGather", AluOp.bypass, replica_groups=groups,
                ins=[agx_in[:]], outs=[agx_out[:]])
            # ---- message-passing matmuls
            use_ctr = [0]

            def mm_phase(ic, c0, cw, ts, join):
                ps = pspool.tile([128, 2048], dt.float32,
                                 name=f"ps_{join}_{ic}_{c0}",
                                 tag=f"bk{(ic * 3 + c0 // 2048) % 2}")
                nsl = (cw + 511) // 512
                if join:
                    for k in range(nsl):
                        f0 = k * 512
                        fw = min(512, cw - f0)
                        nc.tensor.matmul(ps[:, f0:f0 + fw], id_sb[:, :],
                                         acc[:, ic, c0 + f0:c0 + f0 + fw],
                                         start=True, stop=False)
                for j, t in enumerate(ts):
                    last = j == len(ts) - 1
                    for k in range(nsl):
                        f0 = k * 512
                        fw = min(512, cw - f0)
                        nc.tensor.matmul(
                            ps[:, f0:f0 + fw],
                            adjT_sb[:, t, ic * 128:(ic + 1) * 128],
                            qf[:, t, c0 + f0:c0 + f0 + fw],
                            start=(j == 0 and not join),
                            stop=last)
                if use_ctr[0] % 2 == 0:
                    nc.scalar.activation(acc[:, ic, c0:c0 + cw], ps[:, 0:cw],
                                         ACT.Copy)
                else:
                    nc.vector.tensor_copy(acc[:, ic, c0:c0 + cw], ps[:, 0:cw])
                use_ctr[0] += 1

            for ic in range(NIC):
                for (c0, cw) in CHUNK_USES:
                    mm_phase(ic, c0, cw, PH1_T, join=False)

            vx = agx_out.rearrange("(s t p) c -> p s t c", p=128, t=4)
            nc.scalar.dma_start(sxg[:, :, :, :], vx[:, :, :, :])
            # peer sx = shard0*h + shard1*(1-h)  (pm columns from host)
            nc.vector.tensor_scalar_mul(sx8[:, 4:8, :], sxg[:, 0, :, :],
                                        pm_sb[:, 0:1])
            nc.vector.scalar_tensor_tensor(sx8[:, 4:8, :], sxg[:, 1, :, :],
                                           pm_sb[:, 1:2], sx8[:, 4:8, :],
                                           op0=AluOp.mult, op1=AluOp.add)
            stage1(4, "vector")
            stage1(5, "scalar")
            stage1(6, "vector")
            stage1(7, "scalar")

            # ---- phase 2: identity-join + m-tiles {2,3,6,7}, then DMA out
            for ic in range(NIC):
                for (c0, cw) in CHUNK_USES:
                    mm_phase(ic, c0, cw, PH2_T, join=True)
                nc.sync.dma_start(h_out[ic * 128:(ic + 1) * 128, :],
                                  acc[:, ic, :])
    nc.compile()
    return nc


_programs = {}


def _get_program():
    if "v2" not in _programs:
        _programs["v2"] = build()
    return _programs["v2"]


# ---------------------------------------------------------------- host driver
def kernel(v0, v1, v2, s0, s1, s2, conn, _trace=False, _results=None):
    v9 = np.concatenate([np.asarray(v0, np.float32),
                         np.asarray(v1, np.float32),
                         np.asarray(v2, np.float32)], axis=2).astype(BF16)
    s9 = np.concatenate(
        [np.asarray(s, np.float32)[..., 0] for s in (s0, s1, s2)],
        axis=3).astype(BF16)                                 # [B, m, j, 9]
    adjT = np.asarray(conn).transpose(0, 2, 1).astype(BF16)  # [B, m, i]
    ident = np.eye(128, dtype=np.float32).astype(BF16)

    core_ids = list(range(NCORES))
    in_maps = []
    for k in core_ids:
        b, h = divmod(k, 2)
        isl = slice(h * HALF, (h + 1) * HALF)
        own = slice(h * HALF, (h + 1) * HALF)
        peer = slice((1 - h) * HALF, (2 - h) * HALF)
        pm = np.zeros((128, 2), np.float32)
        pm[:, 0] = h            # coefficient of rank-0 shard for the peer half
        pm[:, 1] = 1 - h        # coefficient of rank-1 shard
        in_maps.append({
            "sh": np.ascontiguousarray(s9[b, own]),
            "v9": np.ascontiguousarray(
                np.concatenate([v9[b, own], v9[b, peer]], axis=0)),
            "adjT": np.ascontiguousarray(
                np.concatenate([adjT[b, own, isl], adjT[b, peer, isl]],
                               axis=0)),
            "ident": ident,
            "pm": pm,
        })

    r = run_bass_kernel_spmd(_get_program(), in_maps, core_ids, trace=_trace)
    h_all = np.empty((B, N, NPROD, C), np.float32)
    for k in core_ids:
        b, hh = divmod(k, 2)
        h_all[b, hh * HALF:(hh + 1) * HALF] = (
            r.results[k]["h"].astype(np.float32).reshape(HALF, NPROD, C))

    if _results is not None:
        _results.append(r)

    # host epilogue: fixed CG mix + per-degree normalization
    mp = np.einsum("rp,bipc->birc", CGM, h_all).reshape(B, N, NCH)
    out = np.empty_like(mp)
    for L, (c0, c1) in enumerate(L_RANGES):
        seg = mp[:, :, c0:c1]
        nf = (2 * L + 1) * np.linalg.norm(seg.astype(np.float64))
        out[:, :, c0:c1] = (seg.astype(np.float64) / (nf / C)).astype(np.float32)
    return out
